# revision 12
# baseline (speedup 1.0000x reference)
"""Multi-head attention (no mask) Trainium2 kernel, SPMD over 8 NeuronCores.

Problem: x[2,2048,1024] @ wq/wk/wv[1024,1024] (+zero biases) -> 16-head
scaled-dot-product attention (softmax over full sequence, no causal mask),
output [2,2048,1024] fp32.

Sharding: tensor-parallel over heads. Each core handles 2 heads (128 output
columns) for both batches: per-core weights are the 128-column slice of
wq/wk/wv; per-core output is out[:, :, c0:c0+128]. Host concatenates.

v2 pipeline per core (all matmuls bf16, fp32 PSUM accumulate):
  1. x fp32 DRAM --gpsimd SWDGE cast DMA (chunked)--> x16n SBUF natural
     [128 tok, 4, 1024] bf16 per 512-token chunk (no DRAM round trip).
  2. x16n --SBUF->SBUF HWDGE DMA transpose (SP/Act queues)--> xT
     [128 D-part, 8 t, 2048 tok] bf16.
  3. Weights: fp32 via SP queue -> DVE cast to bf16 (Pool queue stays free
     for the x casts).
  4. qT/kT/vT [128, 2048] = w.T @ xT (+bias on DVE).
  5. v65 via PE transpose: [128 kseq, 16 kc, 130]: [0:64]=v_h0, [64]=1,
     [65:129]=v_h1, [129]=1.
  6. Attention per unit (b, qc of 512): scores row-tiled on the PE (heads on
     the two 64-row tile groups), exp on ScalarE into att[128, 16, 512] bf16;
     AV with att STATIONARY and v65 MOVING (N=65 per matmul): psum
     yacc[128 q, 4 j, 65] accumulates y in natural layout (col 64 = softmax
     denominator), so no final PE transposes. Finalize = DVE reciprocal+mul.
  7. Units are software-pipelined: unit u's AV matmuls interleave into unit
     u+1's score groups so ScalarE (the bottleneck, ~133us of exp) never
     waits; batch 1's projections drain through hook slots.
"""

import os
import sys

import numpy as np

for _p in ("/opt/trn_rl_repo", "/root/.axon_site/_ro/trn_rl_repo"):
    if _p not in sys.path and os.path.isdir(_p):
        sys.path.append(_p)

from contextlib import ExitStack

import concourse.bass as bass
import concourse.tile as tile
from concourse import bacc, masks, mybir
from concourse.bass_utils import run_bass_kernel_spmd

FP32 = mybir.dt.float32
BF16 = mybir.dt.bfloat16

N_CORES = 8
B, S, D = 2, 2048, 1024
COLS = 128            # output columns per core = 2 heads x 64
HD = 64               # head dim
SCALE = 0.125         # 1 / sqrt(HD)
QCH = 512             # q chunk (one attention unit)
KCH = 128             # k chunk (partition dim)
NKC = S // KCH        # 16
NQC = S // QCH        # 4
NJ = QCH // 128       # 4 q sub-blocks per unit
DT = D // 128         # 8 contraction tiles for projections
NSC = S // QCH        # 4 token chunks per batch (same as NQC)
NG = NKC // 2         # 8 score groups (2 kc each) per unit

_CACHED_NC = None


def build_nc(reps=1):
    nc = bacc.Bacc("TRN2", target_bir_lowering=False, debug=False,
                   num_devices=N_CORES)

    x = nc.dram_tensor("x", [B, S, D], FP32, kind="ExternalInput").ap()
    w_ap = {}
    b_ap = {}
    for p in ("q", "k", "v"):
        w_ap[p] = nc.dram_tensor(f"w{p}", [D, COLS], FP32,
                                 kind="ExternalInput").ap()
        b_ap[p] = nc.dram_tensor(f"b{p}", [COLS], FP32,
                                 kind="ExternalInput").ap()
    out = nc.dram_tensor("out", [B, S, COLS], FP32, kind="ExternalOutput").ap()

    with tile.TileContext(nc) as tc, ExitStack() as ctx:
        const_pool = ctx.enter_context(tc.tile_pool(name="const", bufs=1))
        wst_pool = ctx.enter_context(tc.tile_pool(name="wst", bufs=2))
        w_pool = ctx.enter_context(tc.tile_pool(name="w", bufs=1))
        x16_pool = ctx.enter_context(tc.tile_pool(name="x16", bufs=2))
        xt_pool = ctx.enter_context(tc.tile_pool(name="xt", bufs=2))
        qkv_pool = ctx.enter_context(tc.tile_pool(name="qkv", bufs=2))
        v65_pool = ctx.enter_context(tc.tile_pool(name="v65", bufs=2))
        att_pool = ctx.enter_context(tc.tile_pool(name="att", bufs=4))
        fin_pool = ctx.enter_context(tc.tile_pool(name="fin", bufs=4))
        yout_pool = ctx.enter_context(tc.tile_pool(name="yout", bufs=2))
        ps_a = ctx.enter_context(tc.tile_pool(name="psa", bufs=3,
                                              space="PSUM"))
        ps_b = ctx.enter_context(tc.tile_pool(name="psb", bufs=2,
                                              space="PSUM"))

        id_bf16 = const_pool.tile([128, 128], BF16, tag="idb")
        masks.make_identity(nc, id_bf16[:])

        # Weights: fp32 stage via SP queue, DVE cast to bf16. Biases via SP.
        # k first: the b0 prologue's critical chain is cast -> xpose -> k-proj
        # -> scores, so wk's DMA must be first on the shared DMA bus.
        w_sb = {}
        b_sb = {}
        for p in ("k", "q", "v"):
            wst = wst_pool.tile([128, DT, COLS], FP32, tag="wst", name="wst")
            nc.sync.dma_start(
                out=wst[:],
                in_=w_ap[p].rearrange("(t p) c -> p t c", p=128))
            wt = w_pool.tile([128, DT, COLS], BF16, tag=f"w{p}")
            nc.vector.tensor_copy(wt[:], wst[:])
            w_sb[p] = wt
            bt = w_pool.tile([COLS, 1], FP32, tag=f"b{p}")
            nc.sync.dma_start(out=bt[:],
                              in_=b_ap[p].rearrange("(p one) -> p one", one=1))
            b_sb[p] = bt

        state = {}  # per-rep tiles; cleared each rep

        def emit_cast(b, sc):
            """x fp32 DRAM -> x16n bf16 SBUF (natural), one 512-token chunk."""
            t_ = x16_pool.tile([128, NJ, D], BF16, tag="x16n", name="x16n")
            nc.gpsimd.dma_start(
                out=t_[:],
                in_=x[b, sc * QCH:(sc + 1) * QCH, :].rearrange(
                    "(j p) d -> p j d", p=128))
            state[b, "x16", sc] = t_

        def emit_xpose(b, sc, engs=(None, None)):
            """x16n chunk -> xT[b] via SBUF->SBUF HWDGE DMA transposes."""
            if (b, "xt") not in state:
                state[b, "xt"] = xt_pool.tile([128, DT, S], BF16, tag="xt",
                                              name="xt")
            xt = state[b, "xt"]
            x16n = state[b, "x16", sc]
            e0 = engs[0] or nc.sync
            e1 = engs[1] or nc.scalar
            i = 0
            for j in range(NJ):
                for t in range(DT):
                    eng = e0 if (i % 2 == 0) else e1
                    i += 1
                    eng.dma_start(
                        out=xt[:, t,
                               sc * QCH + j * 128:sc * QCH + (j + 1) * 128],
                        in_=x16n[:, j, t * 128:(t + 1) * 128],
                        transpose=True)

        def emit_proj(b, p, sc):
            """One projection for one 512-token chunk: psum += w.T @ xT."""
            if (b, p) not in state:
                state[b, p] = qkv_pool.tile([128, S], BF16, tag=f"{p}T",
                                            name=f"{p}T")
            pt = state[b, p]
            xt = state[b, "xt"]
            ps = ps_a.tile([128, QCH], FP32, tag="A", name="psproj")
            for t in range(DT):
                nc.tensor.matmul(
                    ps[:], lhsT=w_sb[p][:, t, :],
                    rhs=xt[:, t, sc * QCH:(sc + 1) * QCH],
                    start=(t == 0), stop=(t == DT - 1))
            nc.vector.tensor_scalar_add(
                pt[:, sc * QCH:(sc + 1) * QCH], ps[:], b_sb[p][:])

        def emit_v65(b, kc0, n):
            """PE-transpose vT into v-natural with ones columns appended."""
            if (b, "v65") not in state:
                v65 = v65_pool.tile([128, NKC, 130], BF16, tag="v65",
                                    name="v65")
                nc.vector.memset(v65[:, :, 64], 1.0)
                nc.vector.memset(v65[:, :, 129], 1.0)
                state[b, "v65"] = v65
            v65 = state[b, "v65"]
            for kc in range(kc0, kc0 + n):
                pvt = ps_a.tile([128, 128], BF16, tag="A", name="psvt")
                nc.tensor.transpose(pvt[:],
                                    state[b, "v"][:, kc * 128:(kc + 1) * 128],
                                    id_bf16[:])
                nc.vector.tensor_copy(v65[:, kc, 0:64], pvt[:, 0:64])
                nc.vector.tensor_copy(v65[:, kc, 65:129], pvt[:, 64:128])

        def emit_scores_group(u, g):
            """2 kc x 2 heads of scoresT (row-tiled) + exp into att tiles."""
            b, qc = u
            if (u, "att", 0) not in state:
                for h in (0, 1):
                    state[u, "att", h] = att_pool.tile(
                        [128, NKC, QCH], BF16, tag="att", name="att")
            qT, kT = state[b, "q"], state[b, "k"]
            for h in (0, 1):
                pss = ps_a.tile([128, 2, QCH], FP32, tag="A", name="pss")
                for j2 in (0, 1):
                    kc = 2 * g + j2
                    nc.tensor.matmul(
                        pss[:, j2, :],
                        lhsT=kT[h * HD:(h + 1) * HD,
                                kc * 128:(kc + 1) * 128],
                        rhs=qT[h * HD:(h + 1) * HD,
                               qc * QCH:(qc + 1) * QCH],
                        start=True, stop=True)
                nc.scalar.activation(
                    state[u, "att", h][:, 2 * g:2 * g + 2, :], pss[:],
                    mybir.ActivationFunctionType.Exp, scale=SCALE)

        def emit_av_chunk(u, c):
            """One j-block of AV for one head: 16 accumulating matmuls with
            att stationary (lhsT) and v65 moving (N=65) -> y natural."""
            b, qc = u
            h, j = c // NJ, c % NJ
            if (u, "yacc", h) not in state:
                state[u, "yacc", h] = ps_b.tile([128, NJ, 65], FP32, tag="B",
                                                name="yacc")
            yacc = state[u, "yacc", h]
            att = state[u, "att", h]
            v65 = state[b, "v65"]
            for kc in range(NKC):
                nc.tensor.matmul(
                    yacc[:, j, :],
                    lhsT=att[:, kc, j * 128:(j + 1) * 128],
                    rhs=v65[:, kc, h * 65:(h + 1) * 65],
                    start=(kc == 0), stop=(kc == NKC - 1))

        def emit_finalize(u, h):
            """Normalize one head's y by the accumulated denominator col."""
            b, qc = u
            yacc = state[u, "yacc", h]
            if (u, "yo") not in state:
                state[u, "yo"] = yout_pool.tile([128, NJ, COLS], FP32,
                                                tag="yo", name="yo")
            yo = state[u, "yo"]
            rc = fin_pool.tile([128, NJ, 1], FP32, tag="rc", name="rc")
            nc.vector.reciprocal(rc[:], yacc[:, :, 64:65])
            for j in range(NJ):
                nc.vector.tensor_scalar_mul(
                    yo[:, j, h * HD:(h + 1) * HD], yacc[:, j, 0:64],
                    rc[:, j, :])
            if h == 1:
                nc.sync.dma_start(
                    out=out[b, qc * QCH:(qc + 1) * QCH, :].rearrange(
                        "(j p) c -> p j c", p=128),
                    in_=yo[:])

        AV_SKEW = 2  # delay prev's AV by this many groups so it never
        # waits on prev's last exps (PE is in-order; a wait starves ScalarE)

        def emit_attn_unit(u, prev, hook):
            """8 score groups of unit u, with prev unit's AV chunks and
            pending-hook work interleaved into the PE stream."""
            for g in range(NG):
                emit_scores_group(u, g)
                if prev is not None and g >= AV_SKEW:
                    emit_av_chunk(prev, g - AV_SKEW)
                    if g - AV_SKEW == NJ - 1:
                        emit_finalize(prev, 0)
                if hook is not None:
                    hook()
            if prev is not None:
                for c in range(NG - AV_SKEW, NG):
                    emit_av_chunk(prev, c)
                emit_finalize(prev, 1)

        def emit_av_tail(u):
            for c in range(2 * NJ):
                emit_av_chunk(u, c)
                if c == NJ - 1:
                    emit_finalize(u, 0)
            emit_finalize(u, 1)

        for _rep in range(reps):
            state.clear()
            # Pool queue: b0's x cast DMAs up front. b1's are emitted after
            # the b0 prologue so their bus transfers don't jump ahead of
            # b0's transposes in the shared DMA-engine FIFO.
            for sc in range(NSC):
                emit_cast(0, sc)

            # b0 prologue interleaved with unit (0,0)'s score groups so
            # ScalarE starts exp'ing as early as possible. v-projections and
            # v65 ride between groups (only needed by AV, one unit later).
            u0 = (0, 0)
            for sc in range(NSC):
                emit_xpose(0, sc, engs=(nc.sync, nc.sync))
                emit_proj(0, "q", sc)
                emit_proj(0, "k", sc)
                emit_scores_group(u0, 2 * sc)
                emit_proj(0, "v", sc)
                emit_scores_group(u0, 2 * sc + 1)
                emit_v65(0, 4 * sc, 4)

            # b1 transposes on the SP queue (waits ride the SP sequencer).
            for sc in range(NSC):
                emit_cast(1, sc)
            for sc in range(NSC):
                emit_xpose(1, sc, engs=(nc.sync, nc.sync))

            # b1 projections + v65 drain through hook slots during b0's
            # attention units.
            pending = []
            for sc in range(NSC):
                pending.append(lambda sc=sc: emit_proj(1, "q", sc))
                pending.append(lambda sc=sc: emit_proj(1, "k", sc))
            for sc in range(NSC):
                pending.append(lambda sc=sc: emit_proj(1, "v", sc))
                pending.append(lambda sc=sc: emit_v65(1, 4 * sc, 4))
            pending.reverse()

            def hook():
                if pending:
                    pending.pop()()

            units = [(0, 1), (0, 2), (0, 3), (1, 0), (1, 1), (1, 2), (1, 3)]
            prev = u0
            for u in units:
                emit_attn_unit(u, prev, hook)
                prev = u
            while pending:
                pending.pop()()
            emit_av_tail(prev)

    nc.compile()
    return nc


def get_nc():
    global _CACHED_NC
    if _CACHED_NC is None:
        _CACHED_NC = build_nc()
    return _CACHED_NC


def make_in_maps(x, wq, bq, wk, bk, wv, bv):
    in_maps = []
    for i in range(N_CORES):
        c0 = i * COLS
        in_maps.append({
            "x": np.ascontiguousarray(x, dtype=np.float32),
            "wq": np.ascontiguousarray(wq[:, c0:c0 + COLS], dtype=np.float32),
            "wk": np.ascontiguousarray(wk[:, c0:c0 + COLS], dtype=np.float32),
            "wv": np.ascontiguousarray(wv[:, c0:c0 + COLS], dtype=np.float32),
            "bq": np.ascontiguousarray(bq[c0:c0 + COLS], dtype=np.float32),
            "bk": np.ascontiguousarray(bk[c0:c0 + COLS], dtype=np.float32),
            "bv": np.ascontiguousarray(bv[c0:c0 + COLS], dtype=np.float32),
        })
    return in_maps


def kernel(x, wq, bq, wk, bk, wv, bv):
    nc = get_nc()
    in_maps = make_in_maps(x, wq, bq, wk, bk, wv, bv)
    res = run_bass_kernel_spmd(nc, in_maps, list(range(N_CORES)))
    parts = [res.results[i]["out"] for i in range(N_CORES)]
    out = np.concatenate(parts, axis=2).astype(np.float32)
    kernel.last_results = res
    return out


# revision 35
# speedup vs baseline: 3.6108x; 3.6108x over previous
"""Multi-head attention (no mask) Trainium2 kernel, SPMD over 8 NeuronCores.

Problem: x[2,2048,1024] @ wq/wk/wv[1024,1024] (+zero biases) -> 16-head
scaled-dot-product attention (softmax over full sequence, no causal mask),
output [2,2048,1024] fp32.

Sharding: tensor-parallel over heads. Each core handles 2 heads (128 output
columns) for both batches: per-core weights are the 128-column slice of
wq/wk/wv; per-core output is out[:, :, c0:c0+128]. Host concatenates.

v2 pipeline per core (all matmuls bf16, fp32 PSUM accumulate):
  1. x fp32 DRAM --gpsimd SWDGE cast DMA (chunked)--> x16n SBUF natural
     [128 tok, 4, 1024] bf16 per 512-token chunk (no DRAM round trip).
  2. x16n -> xT [128 D-part, 8 t, 2048 tok] bf16 via PE transposes (53ns per
     [128,128] tile; DVE evacuates psum). DMA transposes measured ~1.2us
     per instruction on HW -- 256 of them serialized the whole kernel.
  3. Weights: fp32 via SP queue -> DVE cast to bf16 (Pool queue stays free
     for the x casts).
  4. qT/kT/vT [128, 2048] = w.T @ xT (+bias on DVE).
  5. v65 via PE transpose: [128 kseq, 16 kc, 130]: [0:64]=v_h0, [64]=1,
     [65:129]=v_h1, [129]=1.
  6. Attention per unit (b, qc of 512): scores row-tiled on the PE (heads on
     the two 64-row tile groups via inferred tile_position), exp on ScalarE
     (the bottleneck engine, ~134us total) into att[128, 16, 512] bf16.
     AV_NAT (default): AV with att STATIONARY and v65 MOVING (N=65): psum
     yacc[128 q, 4 j, 65] accumulates y in natural layout (col 64 = softmax
     denominator), no final transposes; finalize = DVE reciprocal+mul.
     AV_NAT=0: v65 stationary, att moving (N=512) -> yT, PE-transposed in
     finalize. Both measure ~equal on HW (~253us slope); NAT sims faster.
  7. Units are software-pipelined: unit u's AV chunks interleave into unit
     u+1's score groups (skewed 2 groups so they never wait on u's last
     exps); batch 1's prologue drains through per-group hook slots.
"""

import os
import sys

import numpy as np

for _p in ("/opt/trn_rl_repo", "/root/.axon_site/_ro/trn_rl_repo"):
    if _p not in sys.path and os.path.isdir(_p):
        sys.path.append(_p)

from contextlib import ExitStack

import concourse.bass as bass
import concourse.tile as tile
from concourse import bacc, masks, mybir
from concourse.bass_utils import run_bass_kernel_spmd

FP32 = mybir.dt.float32
BF16 = mybir.dt.bfloat16

N_CORES = 8
AV_NAT = os.environ.get("AV_NAT", "1") == "1"
B, S, D = 2, 2048, 1024
COLS = 128            # output columns per core = 2 heads x 64
HD = 64               # head dim
SCALE = 0.125         # 1 / sqrt(HD)
QCH = 512             # q chunk (one attention unit)
KCH = 128             # k chunk (partition dim)
NKC = S // KCH        # 16
NQC = S // QCH        # 4
NJ = QCH // 128       # 4 q sub-blocks per unit
DT = D // 128         # 8 contraction tiles for projections
NSC = S // QCH        # 4 token chunks per batch (same as NQC)
NG = NKC // 2         # 8 score groups (2 kc each) per unit

_CACHED_NC = None


def build_nc(reps=1):
    nc = bacc.Bacc("TRN2", target_bir_lowering=False, debug=False,
                   num_devices=N_CORES)

    x = nc.dram_tensor("x", [B, S, D], FP32, kind="ExternalInput").ap()
    w_ap = {}
    b_ap = {}
    for p in ("q", "k", "v"):
        w_ap[p] = nc.dram_tensor(f"w{p}", [D, COLS], FP32,
                                 kind="ExternalInput").ap()
        b_ap[p] = nc.dram_tensor(f"b{p}", [COLS], FP32,
                                 kind="ExternalInput").ap()
    out = nc.dram_tensor("out", [B, S, COLS], FP32, kind="ExternalOutput").ap()

    with tile.TileContext(nc) as tc, ExitStack() as ctx:
        const_pool = ctx.enter_context(tc.tile_pool(name="const", bufs=1))
        wst_pool = ctx.enter_context(tc.tile_pool(name="wst", bufs=2))
        w_pool = ctx.enter_context(tc.tile_pool(name="w", bufs=1))
        x16_pool = ctx.enter_context(tc.tile_pool(name="x16", bufs=2))
        xt_pool = ctx.enter_context(tc.tile_pool(name="xt", bufs=2))
        qkv_pool = ctx.enter_context(tc.tile_pool(name="qkv", bufs=2))
        v65_pool = ctx.enter_context(tc.tile_pool(name="v65", bufs=2))
        att_pool = ctx.enter_context(tc.tile_pool(name="att", bufs=4))
        fin_pool = ctx.enter_context(tc.tile_pool(name="fin", bufs=4))
        yout_pool = ctx.enter_context(tc.tile_pool(name="yout", bufs=2))
        ps_a = ctx.enter_context(tc.tile_pool(name="psa", bufs=3,
                                              space="PSUM"))
        ps_b = ctx.enter_context(tc.tile_pool(name="psb", bufs=2,
                                              space="PSUM"))

        id_bf16 = const_pool.tile([128, 128], BF16, tag="idb")
        masks.make_identity(nc, id_bf16[:])
        id_f32 = const_pool.tile([128, 128], FP32, tag="idf")
        masks.make_identity(nc, id_f32[:])

        # Preload the Exp activation table during the prologue so the first
        # real exp doesn't pay the ~1.3us table load on the critical path.
        warm = const_pool.tile([128, 1], FP32, tag="warm")
        nc.scalar.activation(warm[:], id_f32[:, 0:1],
                             mybir.ActivationFunctionType.Exp, scale=0.125)

        # Weights: fp32 stage via SP queue, DVE cast to bf16. Biases via SP.
        # k first: the b0 prologue's critical chain is cast -> xpose -> k-proj
        # -> scores, so wk's DMA must be first on the shared DMA bus.
        w_sb = {}
        b_sb = {}
        for p in ("k", "q", "v"):
            wst = wst_pool.tile([128, DT, COLS], FP32, tag="wst", name="wst")
            nc.sync.dma_start(
                out=wst[:],
                in_=w_ap[p].rearrange("(t p) c -> p t c", p=128))
            wt = w_pool.tile([128, DT, COLS], BF16, tag=f"w{p}")
            nc.vector.tensor_copy(wt[:], wst[:])
            w_sb[p] = wt
            bt = w_pool.tile([COLS, 1], FP32, tag=f"b{p}")
            nc.sync.dma_start(out=bt[:],
                              in_=b_ap[p].rearrange("(p one) -> p one", one=1))
            b_sb[p] = bt

        state = {}  # per-rep tiles; cleared each rep

        def emit_cast(b, sc):
            """x fp32 DRAM -> x16n bf16 SBUF (natural), one 512-token chunk."""
            t_ = x16_pool.tile([128, NJ, D], BF16, tag="x16n", name="x16n")
            nc.gpsimd.dma_start(
                out=t_[:],
                in_=x[b, sc * QCH:(sc + 1) * QCH, :].rearrange(
                    "(j p) d -> p j d", p=128))
            state[b, "x16", sc] = t_

        def emit_xpose(b, sc, engs=None):
            """x16n chunk -> xT[b] via PE transposes (DVE evacuates psum).
            DMA transposes were measured ~1.2us/instruction on HW; the PE
            does a [128,128] bf16 transpose in 53ns and has slack."""
            if (b, "xt") not in state:
                state[b, "xt"] = xt_pool.tile([128, DT, S], BF16, tag="xt",
                                              name="xt")
            for j in range(NJ):
                emit_xpose_j(b, sc, j)

        def emit_xpose_j(b, sc, j):
            if (b, "xt") not in state:
                state[b, "xt"] = xt_pool.tile([128, DT, S], BF16, tag="xt",
                                              name="xt")
            xt = state[b, "xt"]
            x16n = state[b, "x16", sc]
            pxt = ps_a.tile([128, DT, 128], BF16, tag="A", name="pxt")
            for t in range(DT):
                nc.tensor.transpose(
                    pxt[:, t, :],
                    x16n[:, j, t * 128:(t + 1) * 128],
                    id_bf16[:])
            nc.vector.tensor_copy(
                xt[:, :, sc * QCH + j * 128:sc * QCH + (j + 1) * 128],
                pxt[:])

        def emit_proj(b, p, sc):
            """One projection for one 512-token chunk: psum += w.T @ xT."""
            if (b, p) not in state:
                state[b, p] = qkv_pool.tile([128, S], BF16, tag=f"{p}T",
                                            name=f"{p}T")
            pt = state[b, p]
            xt = state[b, "xt"]
            ps = ps_a.tile([128, QCH], FP32, tag="A", name="psproj")
            for t in range(DT):
                nc.tensor.matmul(
                    ps[:], lhsT=w_sb[p][:, t, :],
                    rhs=xt[:, t, sc * QCH:(sc + 1) * QCH],
                    start=(t == 0), stop=(t == DT - 1))
            nc.vector.tensor_scalar_add(
                pt[:, sc * QCH:(sc + 1) * QCH], ps[:], b_sb[p][:])

        def emit_v65(b, kc0, n):
            """PE-transpose vT into v-natural with ones columns appended."""
            if (b, "v65") not in state:
                v65 = v65_pool.tile([128, NKC, 130], BF16, tag="v65",
                                    name="v65")
                nc.vector.memset(v65[:, :, 64], 1.0)
                nc.vector.memset(v65[:, :, 129], 1.0)
                state[b, "v65"] = v65
            v65 = state[b, "v65"]
            for kc in range(kc0, kc0 + n):
                pvt = ps_a.tile([128, 128], BF16, tag="A", name="psvt")
                nc.tensor.transpose(pvt[:],
                                    state[b, "v"][:, kc * 128:(kc + 1) * 128],
                                    id_bf16[:])
                nc.vector.tensor_copy(
                    v65[:, kc, :].rearrange("p (g c) -> p g c", g=2)[:, :, 0:64],
                    pvt[:].rearrange("p (g c) -> p g c", g=2))

        def emit_scores_group(u, g):
            """2 kc x 2 heads of scoresT (row-tiled) + exp into att tiles."""
            b, qc = u
            if (u, "att", 0) not in state:
                for h in (0, 1):
                    state[u, "att", h] = att_pool.tile(
                        [128, NKC, QCH], BF16, tag="att", name="att")
            qT, kT = state[b, "q"], state[b, "k"]
            for h in (0, 1):
                pss = ps_a.tile([128, 2, QCH], FP32, tag="A", name="pss")
                for j2 in (0, 1):
                    kc = 2 * g + j2
                    nc.tensor.matmul(
                        pss[:, j2, :],
                        lhsT=kT[h * HD:(h + 1) * HD,
                                kc * 128:(kc + 1) * 128],
                        rhs=qT[h * HD:(h + 1) * HD,
                               qc * QCH:(qc + 1) * QCH],
                        start=True, stop=True)
                nc.scalar.activation(
                    state[u, "att", h][:, 2 * g:2 * g + 2, :], pss[:],
                    mybir.ActivationFunctionType.Exp, scale=SCALE)

        def emit_av_chunk(u, c):
            """One chunk of AV for one head (8 chunks per unit).

            AV_NAT: att stationary (lhsT), v65 moving (N=65) -> y natural,
            chunk = one j-block's full 16-kc accumulation. Costs a [128,128]
            stationary load per 65 moving columns.
            Otherwise (default): v65 stationary, att moving (N=512) -> yT,
            chunk = 4 kc of one head; one stationary load per 512 columns
            (finalize then PE-transposes yT).
            """
            b, qc = u
            v65 = state[b, "v65"]
            if AV_NAT:
                h, j = c // NJ, c % NJ
                if (u, "yacc", h) not in state:
                    state[u, "yacc", h] = ps_b.tile([128, NJ, 65], FP32,
                                                    tag="B", name="yacc")
                yacc = state[u, "yacc", h]
                att = state[u, "att", h]
                for kc in range(NKC):
                    nc.tensor.matmul(
                        yacc[:, j, :],
                        lhsT=att[:, kc, j * 128:(j + 1) * 128],
                        rhs=v65[:, kc, h * 65:(h + 1) * 65],
                        start=(kc == 0), stop=(kc == NKC - 1))
            else:
                h, kq = c // NJ, c % NJ
                if (u, "psy", h) not in state:
                    state[u, "psy", h] = ps_b.tile([65, QCH], FP32,
                                                   tag="B", name="psy")
                psy = state[u, "psy", h]
                att = state[u, "att", h]
                for kc in range(4 * kq, 4 * kq + 4):
                    nc.tensor.matmul(
                        psy[:],
                        lhsT=v65[:, kc, h * 65:(h + 1) * 65],
                        rhs=att[:, kc, :],
                        start=(kc == 0), stop=(kc == NKC - 1))

        def emit_finalize(u, h):
            """Normalize one head's y by the accumulated denominator col."""
            b, qc = u
            if (u, "yo") not in state:
                state[u, "yo"] = yout_pool.tile([128, NJ, COLS], FP32,
                                                tag="yo", name="yo")
            yo = state[u, "yo"]
            if AV_NAT:
                yacc = state[u, "yacc", h]
                rc = fin_pool.tile([128, NJ, 1], FP32, tag="rc", name="rc")
                nc.vector.reciprocal(rc[:], yacc[:, :, 64:65])
                for j in range(NJ):
                    nc.vector.tensor_scalar_mul(
                        yo[:, j, h * HD:(h + 1) * HD], yacc[:, j, 0:64],
                        rc[:, j, :])
            else:
                psy = state[u, "psy", h]
                ysb = fin_pool.tile([65, QCH], FP32, tag="ysb", name="ysb")
                nc.vector.tensor_copy(ysb[:], psy[:])
                for j in range(NJ):
                    pyt = ps_a.tile([128, 65], FP32, tag="A", name="psyt")
                    nc.tensor.transpose(pyt[:], ysb[:, j * 128:(j + 1) * 128],
                                        id_f32[0:65, 0:65])
                    rc = fin_pool.tile([128, 1], FP32, tag="rc", name="rc")
                    nc.vector.reciprocal(rc[:], pyt[:, 64:65])
                    nc.vector.tensor_scalar_mul(
                        yo[:, j, h * HD:(h + 1) * HD], pyt[:, 0:64], rc[:])
            if h == 1:
                nc.sync.dma_start(
                    out=out[b, qc * QCH:(qc + 1) * QCH, :].rearrange(
                        "(j p) c -> p j c", p=128),
                    in_=yo[:])

        AV_SKEW = 2  # delay prev's AV by this many groups so it never
        # waits on prev's last exps (PE is in-order; a wait starves ScalarE)

        def emit_attn_unit(u, prev, hook):
            """8 score groups of unit u, with prev unit's AV chunks and
            pending-hook work interleaved into the PE stream."""
            for g in range(NG):
                emit_scores_group(u, g)
                if prev is not None and g >= AV_SKEW:
                    emit_av_chunk(prev, g - AV_SKEW)
                    if g - AV_SKEW == NJ - 1:
                        emit_finalize(prev, 0)
                if hook is not None:
                    hook()
            if prev is not None:
                for c in range(NG - AV_SKEW, NG):
                    emit_av_chunk(prev, c)
                emit_finalize(prev, 1)

        def emit_av_tail(u):
            for c in range(2 * NJ):
                emit_av_chunk(u, c)
                if c == NJ - 1:
                    emit_finalize(u, 0)
            emit_finalize(u, 1)

        for _rep in range(reps):
            state.clear()
            # Pool queue: b0's x cast DMAs up front. b1's are emitted after
            # the b0 prologue so their bus transfers don't jump ahead of
            # b0's transposes in the shared DMA-engine FIFO.
            for sc in range(NSC):
                emit_cast(0, sc)

            # b0 prologue interleaved with unit (0,0)'s score groups so
            # ScalarE starts exp'ing as early as possible. v-projections and
            # v65 ride between groups (only needed by AV, one unit later).
            u0 = (0, 0)
            for sc in range(NSC):
                emit_xpose(0, sc)
                emit_proj(0, "q", sc)
                emit_proj(0, "k", sc)
                emit_scores_group(u0, 2 * sc)
                emit_proj(0, "v", sc)
                emit_scores_group(u0, 2 * sc + 1)

            # v65(b0) after the scores-critical prologue: the DVE copies
            # would otherwise throttle the per-chunk pipeline, and AV(u0)
            # only needs v65 one unit later.
            for sc in range(NSC):
                emit_v65(0, 4 * sc, 4)

            for sc in range(NSC):
                emit_cast(1, sc)

            # b1 projections + v65 drain through hook slots during b0's
            # attention units.
            pending = []
            for sc in range(NSC):
                for j in range(NJ):
                    pending.append(lambda sc=sc, j=j: emit_xpose_j(1, sc, j))
                pending.append(lambda sc=sc: emit_proj(1, "q", sc))
                pending.append(lambda sc=sc: emit_proj(1, "k", sc))
            for sc in range(NSC):
                pending.append(lambda sc=sc: emit_proj(1, "v", sc))
                pending.append(lambda sc=sc: emit_v65(1, 4 * sc, 4))
            pending.reverse()

            def hook():
                if pending:
                    pending.pop()()

            units = [(0, 1), (0, 2), (0, 3), (1, 0), (1, 1), (1, 2), (1, 3)]
            prev = u0
            for u in units:
                emit_attn_unit(u, prev, hook)
                prev = u
            while pending:
                pending.pop()()
            emit_av_tail(prev)

    nc.compile()
    return nc


def get_nc():
    global _CACHED_NC
    if _CACHED_NC is None:
        _CACHED_NC = build_nc()
    return _CACHED_NC


def make_in_maps(x, wq, bq, wk, bk, wv, bv):
    in_maps = []
    for i in range(N_CORES):
        c0 = i * COLS
        in_maps.append({
            "x": np.ascontiguousarray(x, dtype=np.float32),
            "wq": np.ascontiguousarray(wq[:, c0:c0 + COLS], dtype=np.float32),
            "wk": np.ascontiguousarray(wk[:, c0:c0 + COLS], dtype=np.float32),
            "wv": np.ascontiguousarray(wv[:, c0:c0 + COLS], dtype=np.float32),
            "bq": np.ascontiguousarray(bq[c0:c0 + COLS], dtype=np.float32),
            "bk": np.ascontiguousarray(bk[c0:c0 + COLS], dtype=np.float32),
            "bv": np.ascontiguousarray(bv[c0:c0 + COLS], dtype=np.float32),
        })
    return in_maps


def kernel(x, wq, bq, wk, bk, wv, bv):
    nc = get_nc()
    in_maps = make_in_maps(x, wq, bq, wk, bk, wv, bv)
    res = run_bass_kernel_spmd(nc, in_maps, list(range(N_CORES)))
    parts = [res.results[i]["out"] for i in range(N_CORES)]
    out = np.concatenate(parts, axis=2).astype(np.float32)
    kernel.last_results = res
    return out
